# revision 12
# baseline (speedup 1.0000x reference)
"""AttentionReadout Trainium2 kernel (8-core data-parallel over the graph axis).

Reference computation (per graph of 64 nodes, D=512, H=8 heads, hd=64):
    qkv = x @ in_proj_w.T + in_proj_b ; q,k,v = split(qkv)
    attn = softmax(q k^T / sqrt(hd)) v          (per head)
    attn_out = attn @ out_proj_w.T + out_proj_b
    gates = sigmoid(attn_out @ gate_w.T + gate_b)
    out[g] = sum_n attn_out[n] * gates[n]

v2 layout (per core: 128 graphs = 8192 nodes, superblocks of 512 nodes):
  - X^T tiles ([d,n], bf16) via DMA-xbar transpose.
  - Q^T,K^T projected in [e,n] orientation; odd heads' rows live at
    partitions 64:127 and are used IN PLACE via tile_position row band 64
    (no SBUF realign DMA).
  - V natural [n,e] + ones column (ctx matmul also yields softmax denom).
  - Scores S^T[m,n] for all 8 heads of a 128-node block in one 2-bank psum
    tile; exp on ScalarE in 2 quadrant ops covering all heads.
  - ctx + rowsum in one 2-bank psum tile; one reciprocal + one multiply.
  - ctx^T via PE transposes; out projection natural [n,e]; gate column via
    w_eff = out_proj_w.T@gw (reusing ctx^T stationary).
  - sigmoid as 0.5*tanh(x/2)+0.5 folded into per-half gate writes into
    per-block gate matrices G_b [128,8] (zeros persistent).
  - readout: 4 accumulating matmuls (G_b^T @ ao) -> [8,512] psum, evac,
    per-superblock DMA straight to the output rows.
"""

import numpy as np
import ml_dtypes

import concourse.bass as bass
import concourse.mybir as mybir
import concourse.tile as tile
from concourse import bacc
from concourse.bass_utils import run_bass_kernel_spmd
from concourse.masks import make_identity

F32 = mybir.dt.float32
BF16 = mybir.dt.bfloat16

N_CORES = 8
D = 512
H = 8
HD = 64
NPG = 64            # nodes per graph
TOTAL = 65536
ROWS = TOTAL // N_CORES      # 8192 nodes per core
GC = ROWS // NPG             # 128 graphs per core
BLK = 128                    # nodes per block (2 graphs)
SBN = 512                    # nodes per superblock (4 blocks, 8 graphs)
NSB = ROWS // SBN            # 16 superblocks
NBLK = SBN // BLK            # 4 blocks per superblock
DC = D // 128                # 4 d-chunks

# module-level switch used by test.py; harness default is no tracing
TRACE = False

try:
    import jax as _jax
    _jax.config.update("jax_compilation_cache_dir", "/tmp/jax_neff_cache")
    _jax.config.update("jax_persistent_cache_min_compile_time_secs", 10)
    _jax.config.update("jax_persistent_cache_min_entry_size_bytes", 0)
except Exception:
    pass


def _build(has_bqk, has_bv, has_bo, has_gb, rows=ROWS, variant=()):
    variant = set(variant)
    reps = 1
    stage = 8
    for _v in variant:
        if _v.startswith("x") and _v[1:].isdigit():
            reps = int(_v[1:])
        if _v.startswith("s") and _v[1:].isdigit():
            stage = int(_v[1:])
    nsb = rows // SBN
    gc = rows // NPG
    nc = bacc.Bacc(None, target_bir_lowering=False, debug=False)

    xbf = nc.dram_tensor("xbf", [rows, D], BF16, kind="ExternalInput")
    wqk = nc.dram_tensor("wqk", [128, DC, 2 * D], BF16, kind="ExternalInput")
    wv = nc.dram_tensor("wv", [128, DC, D], BF16, kind="ExternalInput")
    wo = nc.dram_tensor("wo", [128, DC, D], BF16, kind="ExternalInput")
    weff = nc.dram_tensor("weff", [1, D], BF16, kind="ExternalInput")
    if has_bqk:
        bqk = nc.dram_tensor("bqk", [128, 2 * DC], F32, kind="ExternalInput")
    if has_bv:
        bv = nc.dram_tensor("bv", [1, D], F32, kind="ExternalInput")
    if has_bo:
        bo = nc.dram_tensor("bo", [1, D], F32, kind="ExternalInput")
    if has_gb:
        gbh = nc.dram_tensor("gbh", [1, 1], F32, kind="ExternalInput")
    out = nc.dram_tensor("out", [gc, D], F32, kind="ExternalOutput")

    from contextlib import ExitStack
    with tile.TileContext(nc) as tc, ExitStack() as st:
        consts = st.enter_context(tc.tile_pool(name="consts", bufs=1))
        p_xt = st.enter_context(tc.tile_pool(name="p_xt", bufs=4))
        p_qkt = st.enter_context(tc.tile_pool(name="p_qkt", bufs=2))
        p_v = st.enter_context(tc.tile_pool(name="p_v", bufs=4))
        p_attn = st.enter_context(tc.tile_pool(name="p_attn", bufs=3))
        p_ctx = st.enter_context(tc.tile_pool(name="p_ctx", bufs=6))
        p_lg = st.enter_context(tc.tile_pool(name="p_lg", bufs=3))
        p_small = st.enter_context(tc.tile_pool(name="p_small", bufs=4))
        # PSUM budget (8 banks): a=2, s=2, c=2, rt=2
        ps_a = st.enter_context(tc.tile_pool(name="ps_a", bufs=2, space="PSUM"))
        ps_s = st.enter_context(tc.tile_pool(name="ps_s", bufs=1, space="PSUM"))
        ps_c = st.enter_context(tc.tile_pool(name="ps_c", bufs=1, space="PSUM"))
        ps_rt = st.enter_context(tc.tile_pool(name="ps_rt", bufs=2, space="PSUM"))

        # ---- constants / weights ----
        ident_bf = consts.tile([128, 128], BF16, tag="ident_bf")
        make_identity(nc, ident_bf[:])

        wqk_sb = consts.tile([128, DC, 2 * D], BF16, tag="wqk")
        nc.scalar.dma_start(wqk_sb[:], wqk[:, :, :])
        wv_sb = consts.tile([128, DC, D], BF16, tag="wv")
        nc.scalar.dma_start(wv_sb[:], wv[:, :, :])
        wo_sb = consts.tile([128, DC, D], BF16, tag="wo")
        nc.scalar.dma_start(wo_sb[:], wo[:, :, :])
        weff_row = consts.tile([1, D], BF16, tag="weff_row")
        nc.scalar.dma_start(weff_row[:], weff[:, :])
        weff_bc = consts.tile([128, D], BF16, tag="weff_bc")
        nc.gpsimd.partition_broadcast(weff_bc[:], weff_row[:])

        if has_bqk:
            bqk_sb = consts.tile([128, 2 * DC], F32, tag="bqk")
            nc.sync.dma_start(bqk_sb[:], bqk[:, :])
        if has_bv:
            bv_row = consts.tile([1, D], F32, tag="bv_row")
            nc.sync.dma_start(bv_row[:], bv[:, :])
            bv_full = consts.tile([128, D], F32, tag="bv_full")
            nc.gpsimd.partition_broadcast(bv_full[:], bv_row[:])
        if has_bo:
            bo_row = consts.tile([1, D], F32, tag="bo_row")
            nc.sync.dma_start(bo_row[:], bo[:, :])
            bo_bf = consts.tile([1, D], BF16, tag="bo_bf")
            nc.vector.tensor_copy(bo_bf[:], bo_row[:])
            ones_col = consts.tile([128, 1], BF16, tag="ones_col")
            nc.vector.memset(ones_col[:], 1.0)
        if has_gb:
            gbh_row = consts.tile([1, 1], F32, tag="gbh_row")
            nc.sync.dma_start(gbh_row[:], gbh[:, :])
            gbh_full = consts.tile([128, 1], F32, tag="gbh_full")
            nc.gpsimd.partition_broadcast(gbh_full[:], gbh_row[:])

        # per-block gate matrices: only cols 2b,2b+1 ever written (half
        # columns each); the zero elsewhere persists across superblocks.
        Gb = []
        for b in range(NBLK):
            g = consts.tile([128, 2 * NBLK], BF16, tag=f"G{b}")
            nc.vector.memset(g[:], 0.0)
            Gb.append(g)

        # ---- main loop: per superblock ----
        for sb in list(range(nsb)) * reps:
            r0 = sb * SBN

            # X^T tiles [d_part, dc, n]
            xt = p_xt.tile([128, DC, SBN], BF16, tag="xt")
            for dc in range(DC):
                nc.sync.dma_start_transpose(
                    xt[:, dc, :], xbf[r0:r0 + SBN, dc * 128:(dc + 1) * 128])

            # Q^T,K^T projection: [e, n] orientation, 8 e-chunks of 128
            qkt = p_qkt.tile([128, 8, SBN], BF16, tag="qkt")
            for ec in range(8):
                ps = ps_a.tile([128, SBN], F32, tag="a")
                for dc in range(DC):
                    nc.tensor.matmul(
                        ps[:],
                        wqk_sb[:, dc, ec * 128:(ec + 1) * 128],
                        xt[:, dc, :],
                        start=(dc == 0), stop=(dc == DC - 1))
                if has_bqk:
                    if ec % 2 == 0:
                        nc.vector.tensor_scalar_add(
                            qkt[:, ec, :], ps[:], bqk_sb[:, ec:ec + 1])
                    else:
                        nc.scalar.activation(
                            qkt[:, ec, :], ps[:],
                            mybir.ActivationFunctionType.Identity,
                            bias=bqk_sb[:, ec:ec + 1])
                else:
                    if ec % 2 == 0:
                        nc.vector.tensor_copy(qkt[:, ec, :], ps[:])
                    else:
                        nc.scalar.copy(qkt[:, ec, :], ps[:])

            if stage <= 1:
                continue
            # V projection (natural [n, e]) per block, with ones column
            v_sbs = []
            for b in range(NBLK):
                ps = ps_a.tile([128, SBN], F32, tag="a")
                for dc in range(DC):
                    nc.tensor.matmul(
                        ps[:, 0:D],
                        xt[:, dc, b * 128:(b + 1) * 128],
                        wv_sb[:, dc, :],
                        start=(dc == 0), stop=(dc == DC - 1))
                vt = p_v.tile([128, H, HD + 1], BF16, tag="v")
                pv = ps[:, 0:D].rearrange("p (h c) -> p h c", h=H)
                if has_bv:
                    nc.vector.tensor_tensor(
                        vt[:, :, 0:HD], pv,
                        bv_full[:].rearrange("p (h c) -> p h c", h=H),
                        mybir.AluOpType.add)
                else:
                    if b % 2 == 0:
                        nc.vector.tensor_copy(vt[:, :, 0:HD], pv)
                    else:
                        nc.scalar.copy(vt[:, :, 0:HD], pv)
                nc.vector.memset(vt[:, :, HD:HD + 1], 1.0)
                v_sbs.append(vt)

            if stage <= 2:
                continue
            # per block: attention + gates
            ctx_sbs = []
            for b in range(NBLK):
                n0 = b * 128
                vt = v_sbs[b]

                # scores S^T[m, n], all 8 heads, one 2-bank psum tile
                # head h = 2j + band; band-64 heads write psum bank 1 so
                # concurrently-running sub-array matmuls never share a bank
                pss_t = ps_s.tile([128, 2, SBN], F32, tag="s")
                pss = pss_t[:].rearrange("p g (j n) -> p g j n", j=4)
                for h in range(H):
                    p0 = (h % 2) * 64
                    nc.tensor.matmul(
                        pss[:, h % 2, h // 2, :],
                        qkt[p0:p0 + 64, 4 + h // 2, n0:n0 + 128],
                        qkt[p0:p0 + 64, h // 2, n0:n0 + 128],
                        start=True, stop=True)

                # exp of the two valid 64x64 quadrant sets (scale 1/sqrt(hd))
                attn = p_attn.tile([128, H, BLK], BF16, tag="attn")
                nc.vector.memset(attn[0:64, :, 64:128], 0.0)
                nc.vector.memset(attn[64:128, :, 0:64], 0.0)
                av = attn[:].rearrange("p (j g) n -> p g j n", g=2)
                nc.scalar.activation(
                    av[0:64, :, :, 0:64], pss[0:64, :, :, 0:64],
                    mybir.ActivationFunctionType.Exp, scale=0.125)
                nc.scalar.activation(
                    av[64:128, :, :, 64:128], pss[64:128, :, :, 64:128],
                    mybir.ActivationFunctionType.Exp, scale=0.125)

                if stage <= 3:
                    continue
                # ctx (+rowsum): [n, hd+1] per head, one 2-bank psum tile
                psc_t = ps_c.tile([128, 2, SBN], F32, tag="c")
                psc = psc_t[:, :, 0:4 * (HD + 1)].rearrange(
                    "p g (j c) -> p g j c", c=HD + 1)
                for h in range(H):
                    nc.tensor.matmul(
                        psc[:, h // 4, h % 4, :],
                        attn[:, h, :],
                        vt[:, h, :],
                        start=True, stop=True)
                rr = p_small.tile([128, 2, 4], F32, tag="rr")
                nc.vector.reciprocal(rr[:], psc[:, :, :, HD])
                ctx = p_ctx.tile([128, H, HD], BF16, tag="ctx")
                cv = ctx[:].rearrange("p (g j) c -> p g j c", g=2)
                nc.vector.tensor_tensor(
                    cv[:], psc[:, :, :, 0:HD],
                    rr[:, :, :, None].to_broadcast((128, 2, 4, HD)),
                    mybir.AluOpType.mult)

                if stage <= 4:
                    continue
                # gate logits l[n] = ctx[n,:] . weff  (mult on Pool,
                # free-dim reduce on DVE)
                cflat = ctx[:].rearrange("p h c -> p (h c)")
                lg = p_lg.tile([128, D], BF16, tag="lg")
                nc.gpsimd.tensor_tensor(
                    lg[:], cflat, weff_bc[:], mybir.AluOpType.mult)
                lgs = p_small.tile([128, 1], F32, tag="lgs")
                nc.vector.tensor_reduce(
                    lgs[:], lg[:], mybir.AxisListType.X, mybir.AluOpType.add)

                if stage <= 5:
                    continue
                # gates: sigmoid(x) = 0.5*tanh(x/2) + 0.5, written into the
                # two live half-columns of G_b
                tb = p_small.tile([128, 1], F32, tag="tb")
                nc.scalar.activation(
                    tb[:], lgs[:],
                    mybir.ActivationFunctionType.Tanh,
                    bias=(gbh_full[:] if has_gb else 0.0), scale=0.5)
                nc.vector.tensor_scalar(
                    Gb[b][0:64, 2 * b:2 * b + 1], tb[0:64, :], 0.5, 0.5,
                    mybir.AluOpType.mult, mybir.AluOpType.add)
                nc.vector.tensor_scalar(
                    Gb[b][64:128, 2 * b + 1:2 * b + 2], tb[64:128, :],
                    0.5, 0.5,
                    mybir.AluOpType.mult, mybir.AluOpType.add)

                ctx_sbs.append(ctx)

            if stage <= 7:
                continue
            # gated readout stage 1: R^T[d, g] = sum_b ctx_b^T @ G_b
            # (dc-outer so each psum accumulation group completes before
            # the next one starts)
            psrt = ps_rt.tile([128, DC, 8], F32, tag="rt")
            for dc in range(DC):
                for b in range(NBLK):
                    nc.tensor.matmul(
                        psrt[:, dc, :],
                        ctx_sbs[b][:].rearrange(
                            "p h c -> p (h c)")[:, dc * 128:(dc + 1) * 128],
                        Gb[b][:, :],
                        start=(b == 0), stop=(b == NBLK - 1))
            # stage 2: out[8 graphs, e] = R^T^T @ WoT  (+ sum-gate * bo)
            rt_sb = p_small.tile([128, DC, 8], BF16, tag="rt_sb")
            nc.vector.tensor_copy(rt_sb[:], psrt[:])
            if has_bo:
                sg = p_small.tile([1, 8], F32, tag="sg")
                psg1 = ps_rt.tile([1, 8], F32, tag="rt")
                for b in range(NBLK):
                    nc.tensor.matmul(
                        psg1[:], ones_col[:], Gb[b][:, :],
                        start=(b == 0), stop=(b == NBLK - 1))
                nc.vector.tensor_copy(sg[:], psg1[:])
                sg_bf = p_small.tile([1, 8], BF16, tag="sg_bf")
                nc.vector.tensor_copy(sg_bf[:], sg[:])
            outp = ps_a.tile([8, SBN], F32, tag="a")
            for dc in range(DC):
                nc.tensor.matmul(
                    outp[:], rt_sb[:, dc, :], wo_sb[:, dc, :],
                    start=(dc == 0),
                    stop=(dc == DC - 1 and not has_bo))
            if has_bo:
                nc.tensor.matmul(
                    outp[:], sg_bf[:], bo_bf[:], start=False, stop=True)
            ro = p_small.tile([8, D], F32, tag="ro")
            if sb % 2 == 0:
                nc.vector.tensor_copy(ro[:], outp[:])
            else:
                nc.scalar.copy(ro[:], outp[:])
            nc.scalar.dma_start(out[sb * 8:(sb + 1) * 8, :], ro[:])

    import time as _time
    _t = _time.time()
    nc.compile()
    print(f"[kernel] bacc compile: {_time.time()-_t:.1f}s", flush=True)
    return nc


def kernel(x, batch, in_proj_w, in_proj_b, out_proj_w, out_proj_b,
           gate_w, gate_b):
    x = np.ascontiguousarray(np.asarray(x, dtype=np.float32))
    in_proj_w = np.asarray(in_proj_w, dtype=np.float32)
    in_proj_b = np.asarray(in_proj_b, dtype=np.float32)
    out_proj_w = np.asarray(out_proj_w, dtype=np.float32)
    out_proj_b = np.asarray(out_proj_b, dtype=np.float32)
    gate_w = np.asarray(gate_w, dtype=np.float32)
    gate_b = np.asarray(gate_b, dtype=np.float32)

    # host-side weight prep
    wqkT = in_proj_w[:2 * D].T                              # [512, 1024]
    wqk_h = np.ascontiguousarray(
        wqkT.reshape(DC, 128, 2 * D).transpose(1, 0, 2)).astype(ml_dtypes.bfloat16)
    wvT = in_proj_w[2 * D:].T                               # [512, 512]
    wv_h = np.ascontiguousarray(
        wvT.reshape(DC, 128, D).transpose(1, 0, 2)).astype(ml_dtypes.bfloat16)
    woT = out_proj_w.T                                      # [512, 512]
    wo_h = np.ascontiguousarray(
        woT.reshape(DC, 128, D).transpose(1, 0, 2)).astype(ml_dtypes.bfloat16)
    weff = (out_proj_w.T @ gate_w[0]).astype(np.float32)    # [512]
    weff_h = np.ascontiguousarray(
        weff.reshape(1, D)).astype(ml_dtypes.bfloat16)

    bqk_np = in_proj_b[:2 * D]
    bv_np = in_proj_b[2 * D:]
    gb_eff = float(gate_b[0] + out_proj_b @ gate_w[0])
    has_bqk = bool(np.any(bqk_np))
    has_bv = bool(np.any(bv_np))
    has_bo = bool(np.any(out_proj_b))
    has_gb = gb_eff != 0.0

    import time as _time
    _t = _time.time()
    nc = _build(has_bqk, has_bv, has_bo, has_gb)
    print(f"[kernel] build total: {_time.time()-_t:.1f}s", flush=True)

    in_maps = []
    for c in range(N_CORES):
        m = {
            "xbf": np.ascontiguousarray(
                x[c * ROWS:(c + 1) * ROWS]).astype(ml_dtypes.bfloat16),
            "wqk": wqk_h, "wv": wv_h, "wo": wo_h, "weff": weff_h,
        }
        if has_bqk:
            m["bqk"] = np.ascontiguousarray(
                bqk_np.reshape(2 * DC, 128).T).astype(np.float32)
        if has_bv:
            m["bv"] = bv_np.reshape(1, D).astype(np.float32)
        if has_bo:
            m["bo"] = out_proj_b.reshape(1, D).astype(np.float32)
        if has_gb:
            m["gbh"] = np.array([[0.5 * gb_eff]], dtype=np.float32)
        in_maps.append(m)

    kernel.last_nc = nc
    kernel.last_in_maps = in_maps
    kernel.last_flags = (has_bqk, has_bv, has_bo, has_gb)

    res = run_bass_kernel_spmd(
        nc, in_maps, core_ids=list(range(N_CORES)), trace=TRACE)
    if TRACE:
        kernel.last_exec_time_ns = res.exec_time_ns
        kernel.last_results = res

    return np.concatenate([r["out"] for r in res.results], axis=0)


kernel.last_exec_time_ns = None
kernel.last_results = None
kernel.last_nc = None
kernel.last_in_maps = None


def _make_runner(nc, in_maps):
    """Build a repeat-callable PJRT runner for `nc` with device-resident
    inputs (mirrors bass2jax.run_bass_via_pjrt's multi-core path, minus
    output donation so buffers can be reused across timing iterations)."""
    import jax
    from jax.sharding import Mesh, PartitionSpec, NamedSharding
    from jax.experimental.shard_map import shard_map
    from concourse import bass2jax

    bass2jax.install_neuronx_cc_hook()
    n_cores = len(in_maps)

    partition_name = (
        nc.partition_id_tensor.name if nc.partition_id_tensor else None)
    in_names, out_names, out_avals, zero_outs = [], [], [], []
    for alloc in nc.m.functions[0].allocations:
        if not isinstance(alloc, mybir.MemoryLocationSet):
            continue
        name = alloc.memorylocations[0].name
        if alloc.kind == "ExternalInput":
            if name != partition_name:
                in_names.append(name)
        elif alloc.kind == "ExternalOutput":
            shape = tuple(alloc.tensor_shape)
            dtype = mybir.dt.np(alloc.dtype)
            out_avals.append(jax.core.ShapedArray(shape, dtype))
            out_names.append(name)
            zero_outs.append(np.zeros(shape, dtype))
    all_in_names = in_names + out_names
    if partition_name is not None:
        all_in_names = all_in_names + [partition_name]

    def _body(*args):
        operands = list(args)
        if partition_name is not None:
            operands.append(bass2jax.partition_id_tensor())
        outs = bass2jax._bass_exec_p.bind(
            *operands,
            out_avals=tuple(out_avals),
            in_names=tuple(all_in_names),
            out_names=tuple(out_names),
            lowering_input_output_aliases=(),
            sim_require_finite=True,
            sim_require_nnan=True,
            nc=nc,
        )
        return tuple(outs)

    devices = jax.devices()[:n_cores]
    mesh = Mesh(np.asarray(devices), ("core",))
    nsp = len(in_names) + len(out_names)
    sharded = jax.jit(
        shard_map(_body, mesh=mesh,
                  in_specs=(PartitionSpec("core"),) * nsp,
                  out_specs=(PartitionSpec("core"),) * len(out_names),
                  check_rep=False),
        keep_unused=True,
    )
    sharding = NamedSharding(mesh, PartitionSpec("core"))
    concat_in = [
        np.concatenate([np.asarray(in_maps[c][name]) for c in range(n_cores)], axis=0)
        for name in in_names
    ] + [np.zeros((n_cores * z.shape[0], *z.shape[1:]), z.dtype) for z in zero_outs]
    dev_in = [jax.device_put(a, sharding) for a in concat_in]

    def run_once(block=True):
        outs = sharded(*dev_in)
        if block:
            jax.block_until_ready(outs)
        return outs

    return run_once


def bench(iters=20, warmup=3):
    """Wall-clock timing is dispatch-dominated (~73 ms) and cannot resolve
    device time; kept for compatibility.  Reports x2-x1 marginal."""
    import time
    assert kernel.last_nc is not None, "call kernel() first"

    runner = _make_runner(kernel.last_nc, kernel.last_in_maps)
    nc_x2 = _build(*kernel.last_flags, variant=("x2",))
    runner_x2 = _make_runner(nc_x2, kernel.last_in_maps)

    def measure(run):
        for _ in range(warmup):
            run()
        ts = []
        for _ in range(iters):
            t0 = time.perf_counter()
            run()
            ts.append(time.perf_counter() - t0)
        ts.sort()
        return ts

    ts_k = measure(runner)
    ts_2 = measure(runner_x2)
    exec_ns = (ts_2[0] - ts_k[0]) * 1e9
    return exec_ns, ts_k[0] * 1e9, ts_2[0] * 1e9


# revision 13
# speedup vs baseline: 1.2561x; 1.2561x over previous
"""AttentionReadout Trainium2 kernel (8-core data-parallel over the graph axis).

Reference computation (per graph of 64 nodes, D=512, H=8 heads, hd=64):
    qkv = x @ in_proj_w.T + in_proj_b ; q,k,v = split(qkv)
    attn = softmax(q k^T / sqrt(hd)) v          (per head)
    attn_out = attn @ out_proj_w.T + out_proj_b
    gates = sigmoid(attn_out @ gate_w.T + gate_b)
    out[g] = sum_n attn_out[n] * gates[n]

v2 layout (per core: 128 graphs = 8192 nodes, superblocks of 512 nodes):
  - X^T tiles ([d,n], bf16) via DMA-xbar transpose.
  - Q^T,K^T projected in [e,n] orientation; odd heads' rows live at
    partitions 64:127 and are used IN PLACE via tile_position row band 64
    (no SBUF realign DMA).
  - V natural [n,e] + ones column (ctx matmul also yields softmax denom).
  - Scores S^T[m,n] for all 8 heads of a 128-node block in one 2-bank psum
    tile; exp on ScalarE in 2 quadrant ops covering all heads.
  - ctx + rowsum in one 2-bank psum tile; one reciprocal + one multiply.
  - ctx^T via PE transposes; out projection natural [n,e]; gate column via
    w_eff = out_proj_w.T@gw (reusing ctx^T stationary).
  - sigmoid as 0.5*tanh(x/2)+0.5 folded into per-half gate writes into
    per-block gate matrices G_b [128,8] (zeros persistent).
  - readout: 4 accumulating matmuls (G_b^T @ ao) -> [8,512] psum, evac,
    per-superblock DMA straight to the output rows.
"""

import numpy as np
import ml_dtypes

import concourse.bass as bass
import concourse.mybir as mybir
import concourse.tile as tile
from concourse import bacc
from concourse.bass_utils import run_bass_kernel_spmd
from concourse.masks import make_identity

F32 = mybir.dt.float32
BF16 = mybir.dt.bfloat16

N_CORES = 8
D = 512
H = 8
HD = 64
NPG = 64            # nodes per graph
TOTAL = 65536
ROWS = TOTAL // N_CORES      # 8192 nodes per core
GC = ROWS // NPG             # 128 graphs per core
BLK = 128                    # nodes per block (2 graphs)
SBN = 512                    # nodes per superblock (4 blocks, 8 graphs)
NSB = ROWS // SBN            # 16 superblocks
NBLK = SBN // BLK            # 4 blocks per superblock
DC = D // 128                # 4 d-chunks

# module-level switch used by test.py; harness default is no tracing
TRACE = False

try:
    import jax as _jax
    _jax.config.update("jax_compilation_cache_dir", "/tmp/jax_neff_cache")
    _jax.config.update("jax_persistent_cache_min_compile_time_secs", 10)
    _jax.config.update("jax_persistent_cache_min_entry_size_bytes", 0)
except Exception:
    pass


def _build(has_bqk, has_bv, has_bo, has_gb, rows=ROWS, variant=()):
    variant = set(variant)
    reps = 1
    stage = 8
    for _v in variant:
        if _v.startswith("x") and _v[1:].isdigit():
            reps = int(_v[1:])
        if _v.startswith("s") and _v[1:].isdigit():
            stage = int(_v[1:])
    nsb = rows // SBN
    gc = rows // NPG
    nc = bacc.Bacc(None, target_bir_lowering=False, debug=False)

    xbf = nc.dram_tensor("xbf", [rows, D], BF16, kind="ExternalInput")
    wqk = nc.dram_tensor("wqk", [128, DC, 2 * D], BF16, kind="ExternalInput")
    wv = nc.dram_tensor("wv", [128, DC, D], BF16, kind="ExternalInput")
    wo = nc.dram_tensor("wo", [128, DC, D], BF16, kind="ExternalInput")
    weff = nc.dram_tensor("weff", [1, D], BF16, kind="ExternalInput")
    if has_bqk:
        bqk = nc.dram_tensor("bqk", [128, 2 * DC], F32, kind="ExternalInput")
    if has_bv:
        bv = nc.dram_tensor("bv", [1, D], F32, kind="ExternalInput")
    if has_bo:
        bo = nc.dram_tensor("bo", [1, D], F32, kind="ExternalInput")
    if has_gb:
        gbh = nc.dram_tensor("gbh", [1, 1], F32, kind="ExternalInput")
    out = nc.dram_tensor("out", [gc, D], F32, kind="ExternalOutput")

    from contextlib import ExitStack

    with tile.TileContext(nc) as tc, ExitStack() as st:
        consts = st.enter_context(tc.tile_pool(name="consts", bufs=1))
        p_xt = st.enter_context(tc.tile_pool(name="p_xt", bufs=4))
        p_qkt = st.enter_context(tc.tile_pool(name="p_qkt", bufs=2))
        p_v = st.enter_context(tc.tile_pool(name="p_v", bufs=4))
        p_attn = st.enter_context(tc.tile_pool(name="p_attn", bufs=3))
        p_ctx = st.enter_context(tc.tile_pool(name="p_ctx", bufs=10))
        p_lg = st.enter_context(tc.tile_pool(name="p_lg", bufs=3))
        p_small = st.enter_context(tc.tile_pool(name="p_small", bufs=4))
        # PSUM budget (8 banks): a=2, s=2, c=2, rt=2
        ps_a = st.enter_context(tc.tile_pool(name="ps_a", bufs=2, space="PSUM"))
        ps_s = st.enter_context(tc.tile_pool(name="ps_s", bufs=1, space="PSUM"))
        ps_c = st.enter_context(tc.tile_pool(name="ps_c", bufs=1, space="PSUM"))
        ps_rt = st.enter_context(tc.tile_pool(name="ps_rt", bufs=2, space="PSUM"))

        # ---- constants / weights ----
        ident_bf = consts.tile([128, 128], BF16, tag="ident_bf")
        make_identity(nc, ident_bf[:])

        wqk_sb = consts.tile([128, DC, 2 * D], BF16, tag="wqk")
        nc.scalar.dma_start(wqk_sb[:], wqk[:, :, :])
        wv_sb = consts.tile([128, DC, D], BF16, tag="wv")
        nc.scalar.dma_start(wv_sb[:], wv[:, :, :])
        wo_sb = consts.tile([128, DC, D], BF16, tag="wo")
        nc.scalar.dma_start(wo_sb[:], wo[:, :, :])
        weff_row = consts.tile([1, D], BF16, tag="weff_row")
        nc.scalar.dma_start(weff_row[:], weff[:, :])
        weff_bc = consts.tile([128, D], BF16, tag="weff_bc")
        nc.gpsimd.partition_broadcast(weff_bc[:], weff_row[:])

        if has_bqk:
            bqk_sb = consts.tile([128, 2 * DC], F32, tag="bqk")
            nc.sync.dma_start(bqk_sb[:], bqk[:, :])
        if has_bv:
            bv_row = consts.tile([1, D], F32, tag="bv_row")
            nc.sync.dma_start(bv_row[:], bv[:, :])
            bv_full = consts.tile([128, D], F32, tag="bv_full")
            nc.gpsimd.partition_broadcast(bv_full[:], bv_row[:])
        if has_bo:
            bo_row = consts.tile([1, D], F32, tag="bo_row")
            nc.sync.dma_start(bo_row[:], bo[:, :])
            bo_bf = consts.tile([1, D], BF16, tag="bo_bf")
            nc.vector.tensor_copy(bo_bf[:], bo_row[:])
            ones_col = consts.tile([128, 1], BF16, tag="ones_col")
            nc.vector.memset(ones_col[:], 1.0)
        if has_gb:
            gbh_row = consts.tile([1, 1], F32, tag="gbh_row")
            nc.sync.dma_start(gbh_row[:], gbh[:, :])
            gbh_full = consts.tile([128, 1], F32, tag="gbh_full")
            nc.gpsimd.partition_broadcast(gbh_full[:], gbh_row[:])

        # per-block gate matrices: only cols 2b,2b+1 ever written (half
        # columns each); the zero elsewhere persists across superblocks.
        Gb = []
        for b in range(NBLK):
            g = consts.tile([128, 2 * NBLK], BF16, tag=f"G{b}")
            nc.vector.memset(g[:], 0.0)
            Gb.append(g)

        # ---- main loop: per superblock ----
        pending_readout = []
        for sb in list(range(nsb)) * reps:
            r0 = sb * SBN

            # X^T tiles [d_part, dc, n]
            xt = p_xt.tile([128, DC, SBN], BF16, tag="xt")
            for dc in range(DC):
                eng = nc.sync if dc % 2 == 0 else nc.scalar
                eng.dma_start_transpose(
                    xt[:, dc, :], xbf[r0:r0 + SBN, dc * 128:(dc + 1) * 128])

            # Q^T,K^T projection: [e, n] orientation, 8 e-chunks of 128
            qkt = p_qkt.tile([128, 8, SBN], BF16, tag="qkt")
            for ec in range(8):
                ps = ps_a.tile([128, SBN], F32, tag="a")
                for dc in range(DC):
                    nc.tensor.matmul(
                        ps[:],
                        wqk_sb[:, dc, ec * 128:(ec + 1) * 128],
                        xt[:, dc, :],
                        start=(dc == 0), stop=(dc == DC - 1))
                if has_bqk:
                    if ec % 2 == 0:
                        nc.vector.tensor_scalar_add(
                            qkt[:, ec, :], ps[:], bqk_sb[:, ec:ec + 1])
                    else:
                        nc.scalar.activation(
                            qkt[:, ec, :], ps[:],
                            mybir.ActivationFunctionType.Identity,
                            bias=bqk_sb[:, ec:ec + 1])
                else:
                    if ec % 2 == 0:
                        nc.vector.tensor_copy(qkt[:, ec, :], ps[:])
                    else:
                        nc.scalar.copy(qkt[:, ec, :], ps[:])

            if stage <= 1:
                continue
            # pipelined: previous superblock's readout lands here, after
            # this superblock's qkt matmuls are already in the PE stream
            while pending_readout:
                _sbv, _ctxs = pending_readout.pop(0)
                emit_readout(_sbv, _ctxs)

            # V projection (natural [n, e]) per block, with ones column
            v_sbs = []
            for b in range(NBLK):
                ps = ps_a.tile([128, SBN], F32, tag="a")
                for dc in range(DC):
                    nc.tensor.matmul(
                        ps[:, 0:D],
                        xt[:, dc, b * 128:(b + 1) * 128],
                        wv_sb[:, dc, :],
                        start=(dc == 0), stop=(dc == DC - 1))
                vt = p_v.tile([128, H, HD + 1], BF16, tag="v")
                pv = ps[:, 0:D].rearrange("p (h c) -> p h c", h=H)
                if has_bv:
                    nc.vector.tensor_tensor(
                        vt[:, :, 0:HD], pv,
                        bv_full[:].rearrange("p (h c) -> p h c", h=H),
                        mybir.AluOpType.add)
                else:
                    if b % 2 == 0:
                        nc.vector.tensor_copy(vt[:, :, 0:HD], pv)
                    else:
                        nc.scalar.copy(vt[:, :, 0:HD], pv)
                nc.vector.memset(vt[:, :, HD:HD + 1], 1.0)
                v_sbs.append(vt)

            if stage <= 2:
                continue
            # per block: attention + gates
            ctx_sbs = []
            for b in range(NBLK):
                n0 = b * 128
                vt = v_sbs[b]

                # scores S^T[m, n], all 8 heads, one 2-bank psum tile
                # head h = 2j + band; band-64 heads write psum bank 1 so
                # concurrently-running sub-array matmuls never share a bank
                pss_t = ps_s.tile([128, 2, SBN], F32, tag="s")
                pss = pss_t[:].rearrange("p g (j n) -> p g j n", j=4)
                for h in range(H):
                    p0 = (h % 2) * 64
                    nc.tensor.matmul(
                        pss[:, h % 2, h // 2, :],
                        qkt[p0:p0 + 64, 4 + h // 2, n0:n0 + 128],
                        qkt[p0:p0 + 64, h // 2, n0:n0 + 128],
                        start=True, stop=True)

                # exp of the two valid 64x64 quadrant sets (scale 1/sqrt(hd))
                attn = p_attn.tile([128, H, BLK], BF16, tag="attn")
                nc.vector.memset(attn[0:64, :, 64:128], 0.0)
                nc.vector.memset(attn[64:128, :, 0:64], 0.0)
                av = attn[:].rearrange("p (j g) n -> p g j n", g=2)
                nc.scalar.activation(
                    av[0:64, :, :, 0:64], pss[0:64, :, :, 0:64],
                    mybir.ActivationFunctionType.Exp, scale=0.125)
                nc.scalar.activation(
                    av[64:128, :, :, 64:128], pss[64:128, :, :, 64:128],
                    mybir.ActivationFunctionType.Exp, scale=0.125)

                if stage <= 3:
                    continue
                # ctx (+rowsum): [n, hd+1] per head, one 2-bank psum tile
                psc_t = ps_c.tile([128, 2, SBN], F32, tag="c")
                psc = psc_t[:, :, 0:4 * (HD + 1)].rearrange(
                    "p g (j c) -> p g j c", c=HD + 1)
                for h in range(H):
                    nc.tensor.matmul(
                        psc[:, h // 4, h % 4, :],
                        attn[:, h, :],
                        vt[:, h, :],
                        start=True, stop=True)
                rr = p_small.tile([128, 2, 4], F32, tag="rr")
                nc.vector.reciprocal(rr[:], psc[:, :, :, HD])
                ctx = p_ctx.tile([128, H, HD], BF16, tag="ctx")
                cv = ctx[:].rearrange("p (g j) c -> p g j c", g=2)
                nc.vector.tensor_tensor(
                    cv[:], psc[:, :, :, 0:HD],
                    rr[:, :, :, None].to_broadcast((128, 2, 4, HD)),
                    mybir.AluOpType.mult)

                if stage <= 4:
                    continue
                # gate logits l[n] = ctx[n,:] . weff  (mult on Pool,
                # free-dim reduce on DVE)
                cflat = ctx[:].rearrange("p h c -> p (h c)")
                lg = p_lg.tile([128, D], BF16, tag="lg")
                nc.gpsimd.tensor_tensor(
                    lg[:], cflat, weff_bc[:], mybir.AluOpType.mult)
                lgs = p_small.tile([128, 1], F32, tag="lgs")
                nc.vector.tensor_reduce(
                    lgs[:], lg[:], mybir.AxisListType.X, mybir.AluOpType.add)

                if stage <= 5:
                    continue
                # gates: sigmoid(x) = 0.5*tanh(x/2) + 0.5, written into the
                # two live half-columns of G_b
                tb = p_small.tile([128, 1], F32, tag="tb")
                nc.scalar.activation(
                    tb[:], lgs[:],
                    mybir.ActivationFunctionType.Tanh,
                    bias=(gbh_full[:] if has_gb else 0.0), scale=0.5)
                nc.vector.tensor_scalar(
                    Gb[b][0:64, 2 * b:2 * b + 1], tb[0:64, :], 0.5, 0.5,
                    mybir.AluOpType.mult, mybir.AluOpType.add)
                nc.vector.tensor_scalar(
                    Gb[b][64:128, 2 * b + 1:2 * b + 2], tb[64:128, :],
                    0.5, 0.5,
                    mybir.AluOpType.mult, mybir.AluOpType.add)

                ctx_sbs.append(ctx)

            if stage <= 7:
                continue

            def emit_readout(sbv, ctxs):
                # stage 1: R^T[d, g] = sum_b ctx_b^T @ G_b  (dc-outer so
                # each psum accumulation group completes before the next)
                psrt = ps_rt.tile([128, DC, 8], F32, tag="rt")
                for dc in range(DC):
                    for b in range(NBLK):
                        nc.tensor.matmul(
                            psrt[:, dc, :],
                            ctxs[b][:].rearrange(
                                "p h c -> p (h c)")[:,
                                                    dc * 128:(dc + 1) * 128],
                            Gb[b][:, :],
                            start=(b == 0), stop=(b == NBLK - 1))
                # stage 2: out[8 graphs, e] = R^T^T @ WoT (+ sum-gate * bo)
                rt_sb = p_small.tile([128, DC, 8], BF16, tag="rt_sb")
                nc.vector.tensor_copy(rt_sb[:], psrt[:])
                if has_bo:
                    sg_bf = p_small.tile([1, 8], BF16, tag="sg_bf")
                    psg1 = ps_rt.tile([1, 8], F32, tag="rt")
                    for b in range(NBLK):
                        nc.tensor.matmul(
                            psg1[:], ones_col[:], Gb[b][:, :],
                            start=(b == 0), stop=(b == NBLK - 1))
                    nc.vector.tensor_copy(sg_bf[:], psg1[:])
                outp = ps_a.tile([8, SBN], F32, tag="a")
                for dc in range(DC):
                    nc.tensor.matmul(
                        outp[:], rt_sb[:, dc, :], wo_sb[:, dc, :],
                        start=(dc == 0),
                        stop=(dc == DC - 1 and not has_bo))
                if has_bo:
                    nc.tensor.matmul(
                        outp[:], sg_bf[:], bo_bf[:], start=False, stop=True)
                ro = p_small.tile([8, D], F32, tag="ro")
                if sbv % 2 == 0:
                    nc.vector.tensor_copy(ro[:], outp[:])
                else:
                    nc.scalar.copy(ro[:], outp[:])
                nc.scalar.dma_start(out[sbv * 8:(sbv + 1) * 8, :], ro[:])

            pending_readout.append((sb, ctx_sbs))

        while pending_readout:
            _sbv, _ctxs = pending_readout.pop(0)
            emit_readout(_sbv, _ctxs)

    import time as _time
    _t = _time.time()
    nc.compile()
    print(f"[kernel] bacc compile: {_time.time()-_t:.1f}s", flush=True)
    return nc


def kernel(x, batch, in_proj_w, in_proj_b, out_proj_w, out_proj_b,
           gate_w, gate_b):
    x = np.ascontiguousarray(np.asarray(x, dtype=np.float32))
    in_proj_w = np.asarray(in_proj_w, dtype=np.float32)
    in_proj_b = np.asarray(in_proj_b, dtype=np.float32)
    out_proj_w = np.asarray(out_proj_w, dtype=np.float32)
    out_proj_b = np.asarray(out_proj_b, dtype=np.float32)
    gate_w = np.asarray(gate_w, dtype=np.float32)
    gate_b = np.asarray(gate_b, dtype=np.float32)

    # host-side weight prep
    wqkT = in_proj_w[:2 * D].T                              # [512, 1024]
    wqk_h = np.ascontiguousarray(
        wqkT.reshape(DC, 128, 2 * D).transpose(1, 0, 2)).astype(ml_dtypes.bfloat16)
    wvT = in_proj_w[2 * D:].T                               # [512, 512]
    wv_h = np.ascontiguousarray(
        wvT.reshape(DC, 128, D).transpose(1, 0, 2)).astype(ml_dtypes.bfloat16)
    woT = out_proj_w.T                                      # [512, 512]
    wo_h = np.ascontiguousarray(
        woT.reshape(DC, 128, D).transpose(1, 0, 2)).astype(ml_dtypes.bfloat16)
    weff = (out_proj_w.T @ gate_w[0]).astype(np.float32)    # [512]
    weff_h = np.ascontiguousarray(
        weff.reshape(1, D)).astype(ml_dtypes.bfloat16)

    bqk_np = in_proj_b[:2 * D]
    bv_np = in_proj_b[2 * D:]
    gb_eff = float(gate_b[0] + out_proj_b @ gate_w[0])
    has_bqk = bool(np.any(bqk_np))
    has_bv = bool(np.any(bv_np))
    has_bo = bool(np.any(out_proj_b))
    has_gb = gb_eff != 0.0

    import time as _time
    _t = _time.time()
    nc = _build(has_bqk, has_bv, has_bo, has_gb)
    print(f"[kernel] build total: {_time.time()-_t:.1f}s", flush=True)

    in_maps = []
    for c in range(N_CORES):
        m = {
            "xbf": np.ascontiguousarray(
                x[c * ROWS:(c + 1) * ROWS]).astype(ml_dtypes.bfloat16),
            "wqk": wqk_h, "wv": wv_h, "wo": wo_h, "weff": weff_h,
        }
        if has_bqk:
            m["bqk"] = np.ascontiguousarray(
                bqk_np.reshape(2 * DC, 128).T).astype(np.float32)
        if has_bv:
            m["bv"] = bv_np.reshape(1, D).astype(np.float32)
        if has_bo:
            m["bo"] = out_proj_b.reshape(1, D).astype(np.float32)
        if has_gb:
            m["gbh"] = np.array([[0.5 * gb_eff]], dtype=np.float32)
        in_maps.append(m)

    kernel.last_nc = nc
    kernel.last_in_maps = in_maps
    kernel.last_flags = (has_bqk, has_bv, has_bo, has_gb)

    res = run_bass_kernel_spmd(
        nc, in_maps, core_ids=list(range(N_CORES)), trace=TRACE)
    if TRACE:
        kernel.last_exec_time_ns = res.exec_time_ns
        kernel.last_results = res

    return np.concatenate([r["out"] for r in res.results], axis=0)


kernel.last_exec_time_ns = None
kernel.last_results = None
kernel.last_nc = None
kernel.last_in_maps = None


def _make_runner(nc, in_maps):
    """Build a repeat-callable PJRT runner for `nc` with device-resident
    inputs (mirrors bass2jax.run_bass_via_pjrt's multi-core path, minus
    output donation so buffers can be reused across timing iterations)."""
    import jax
    from jax.sharding import Mesh, PartitionSpec, NamedSharding
    from jax.experimental.shard_map import shard_map
    from concourse import bass2jax

    bass2jax.install_neuronx_cc_hook()
    n_cores = len(in_maps)

    partition_name = (
        nc.partition_id_tensor.name if nc.partition_id_tensor else None)
    in_names, out_names, out_avals, zero_outs = [], [], [], []
    for alloc in nc.m.functions[0].allocations:
        if not isinstance(alloc, mybir.MemoryLocationSet):
            continue
        name = alloc.memorylocations[0].name
        if alloc.kind == "ExternalInput":
            if name != partition_name:
                in_names.append(name)
        elif alloc.kind == "ExternalOutput":
            shape = tuple(alloc.tensor_shape)
            dtype = mybir.dt.np(alloc.dtype)
            out_avals.append(jax.core.ShapedArray(shape, dtype))
            out_names.append(name)
            zero_outs.append(np.zeros(shape, dtype))
    all_in_names = in_names + out_names
    if partition_name is not None:
        all_in_names = all_in_names + [partition_name]

    def _body(*args):
        operands = list(args)
        if partition_name is not None:
            operands.append(bass2jax.partition_id_tensor())
        outs = bass2jax._bass_exec_p.bind(
            *operands,
            out_avals=tuple(out_avals),
            in_names=tuple(all_in_names),
            out_names=tuple(out_names),
            lowering_input_output_aliases=(),
            sim_require_finite=True,
            sim_require_nnan=True,
            nc=nc,
        )
        return tuple(outs)

    devices = jax.devices()[:n_cores]
    mesh = Mesh(np.asarray(devices), ("core",))
    nsp = len(in_names) + len(out_names)
    sharded = jax.jit(
        shard_map(_body, mesh=mesh,
                  in_specs=(PartitionSpec("core"),) * nsp,
                  out_specs=(PartitionSpec("core"),) * len(out_names),
                  check_rep=False),
        keep_unused=True,
    )
    sharding = NamedSharding(mesh, PartitionSpec("core"))
    concat_in = [
        np.concatenate([np.asarray(in_maps[c][name]) for c in range(n_cores)], axis=0)
        for name in in_names
    ] + [np.zeros((n_cores * z.shape[0], *z.shape[1:]), z.dtype) for z in zero_outs]
    dev_in = [jax.device_put(a, sharding) for a in concat_in]

    def run_once(block=True):
        outs = sharded(*dev_in)
        if block:
            jax.block_until_ready(outs)
        return outs

    return run_once


def bench(iters=20, warmup=3):
    """Wall-clock timing is dispatch-dominated (~73 ms) and cannot resolve
    device time; kept for compatibility.  Reports x2-x1 marginal."""
    import time
    assert kernel.last_nc is not None, "call kernel() first"

    runner = _make_runner(kernel.last_nc, kernel.last_in_maps)
    nc_x2 = _build(*kernel.last_flags, variant=("x2",))
    runner_x2 = _make_runner(nc_x2, kernel.last_in_maps)

    def measure(run):
        for _ in range(warmup):
            run()
        ts = []
        for _ in range(iters):
            t0 = time.perf_counter()
            run()
            ts.append(time.perf_counter() - t0)
        ts.sort()
        return ts

    ts_k = measure(runner)
    ts_2 = measure(runner_x2)
    exec_ns = (ts_2[0] - ts_k[0]) * 1e9
    return exec_ns, ts_k[0] * 1e9, ts_2[0] * 1e9
